# revision 19
# baseline (speedup 1.0000x reference)
"""Trainium2 Bass kernel for per-sample modulated/demodulated 3x3 conv.

Problem: x (8,512,32,32), s (8,512), w (512,512,3,3) ->
  wm[b,o,i,ky,kx] = w * (s[b,i]+1); demod by rsqrt(sum wm^2 + eps) per (b,o);
  y[b] = conv2d_same(x[b], wm[b]).

Sharding: data-parallel over batch, 1 sample per NeuronCore (8 cores).

Per-core algorithm:
  - modulation folded into x: x'[i,p] = x[i,p] * (1+s_i)  (cheaper than
    scaling w: 1024 elems/chan vs 4608)
  - demodulation folded into the output: y = conv(x', w) * denom[o], with
    denom[o] = 1/sqrt(sum_i (1+s_i)^2 * wsq[i,o] + eps),
    wsq[i,o] = sum_pos w[o,i,pos]^2 (DVE square + contiguous add-tree over
    the 9 positions), contraction over i via a tiny PE matvec into one
    PSUM bank.
  - conv as 9 shifted-window matmuls per (cin_chunk, cout_chunk) pair over a
    zero-padded 34x34 x buffer; accumulated in all 8 PSUM banks
    (4 cout chunks x 2 spatial halves of 512 pixels).
  - matmuls run in bf16 (fp32 PE throughput is 1/4 of bf16 on TRN2); inputs
    arrive fp32 and are cast on device; PSUM accumulation is fp32; bf16*bf16
    products are exact in fp32, so the only error is input rounding (~1e-3).

w is pre-packed host-side to w9[cin_chunk, 128, pos, cout] so the matmul
lhsT (contraction dim = cin on partitions, contiguous cout columns) DMAs
and fast-weight-loads cleanly. The last cin chunk's conv loop is cout-outer
so accumulators finish one cout chunk at a time, letting drains/demod/stores
overlap the remaining matmuls.
"""

import os
import sys

if "/opt/trn_rl_repo" not in sys.path:
    sys.path.insert(0, "/opt/trn_rl_repo")

import numpy as np

B = 8
CIN = 512
COUT = 512
H = 32
W = 32
KPOS = 9  # 3x3 kernel positions
HP = H + 2  # padded
WP = W + 2
NCH = CIN // 128  # cin chunks
OCH = COUT // 128  # cout chunks
EPS = 1e-8

_compiled_nc = None


def _build():
    import concourse.tile as tile
    from concourse import bacc, mybir

    F32 = mybir.dt.float32
    MMDT = F32 if os.environ.get("BASS_CONV_F32") else mybir.dt.bfloat16

    nc = bacc.Bacc("TRN2", target_bir_lowering=False, debug=False, num_devices=B)
    x_d = nc.dram_tensor("x", [CIN, H, W], F32, kind="ExternalInput").ap()
    s_d = nc.dram_tensor("s", [CIN, 1], F32, kind="ExternalInput").ap()
    w9_d = nc.dram_tensor("w9", [NCH, 128, KPOS, COUT], F32, kind="ExternalInput").ap()
    y_d = nc.dram_tensor("y", [COUT, H * W], F32, kind="ExternalOutput").ap()

    with tile.TileContext(nc) as tc:
        with (
            tc.tile_pool(name="stage", bufs=2) as stage,
            tc.tile_pool(name="wpool", bufs=1) as wpool,
            tc.tile_pool(name="xpool", bufs=1) as xpool,
            tc.tile_pool(name="sqpool", bufs=2) as sqpool,
            tc.tile_pool(name="misc", bufs=1) as misc,
            tc.tile_pool(name="ypool", bufs=1) as ypool,
            tc.tile_pool(name="psum", bufs=8, space="PSUM") as psum,
        ):
            w_sb = [
                wpool.tile([128, KPOS, COUT], MMDT, name=f"w_sb{c}", tag=f"w{c}")
                for c in range(NCH)
            ]
            xp = [
                xpool.tile([128, HP, WP], MMDT, name=f"xp{c}", tag=f"x{c}")
                for c in range(NCH)
            ]
            s1 = [
                misc.tile([128, 1], F32, name=f"s1_{c}", tag=f"s1_{c}")
                for c in range(NCH)
            ]
            q = [
                misc.tile([128, 1], MMDT, name=f"q_{c}", tag=f"q_{c}")
                for c in range(NCH)
            ]
            wsq = [
                misc.tile([128, COUT], MMDT, name=f"wsq{c}", tag=f"wsq{c}")
                for c in range(NCH)
            ]
            den_s = misc.tile([128, OCH], F32, name="den_s", tag="den_s")
            den = misc.tile([128, OCH], F32, name="den", tag="den")
            y_sb = [
                ypool.tile([128, H * W], F32, name=f"y_sb{o}", tag=f"y{o}")
                for o in range(OCH)
            ]
            eps_t = misc.tile([128, 1], F32, name="eps_t", tag="eps_t")
            nc.vector.memset(eps_t, EPS)

            # --- PE warmup: dummy matmuls on zeroed scratch while the first
            # DMAs are in flight, so the HAM clock gate is warm (2.4 GHz)
            # when the real matmuls start. The scratch PSUM bank is released
            # before the last conv accumulator needs its slot.
            junk = misc.tile([128, 512], MMDT, name="junk", tag="junk")
            nc.vector.memset(junk, 0.0)
            warm = psum.tile([128, 512], F32, name="warm", tag="acc")
            for _ in range(10):
                nc.tensor.matmul(
                    warm, lhsT=junk[:, 0:128], rhs=junk, start=True, stop=True
                )

            # --- input DMAs + casts + x modulation; chunk-ordered so chunk 0
            # is ready first and conv matmuls start during remaining loads.
            for c in range(NCH):
                # s on the SWDGE path, x on the scalar HWDGE queue, w on the
                # sync HWDGE queue -> all three transfer in parallel.
                nc.gpsimd.dma_start(out=s1[c], in_=s_d[c * 128 : (c + 1) * 128, :])
                # on DVE, not ACT: the scalar engine must stay free for the
                # w casts, which gate the conv matmuls
                nc.vector.tensor_scalar_add(s1[c], s1[c], 1.0)  # 1 + s

                if c == 0:
                    # pos-0 weights gate the very first matmul: issue their
                    # DMA + cast ahead of everything else on sync/ACT
                    ws0 = stage.tile(
                        [128, 1, COUT], F32, name="ws0_first", tag="wstage", bufs=6
                    )
                    nc.sync.dma_start(out=ws0, in_=w9_d[0, :, 0:1, :])
                    nc.scalar.copy(w_sb[0][:, 0, :], ws0[:, 0, :])

                # x in two row-halves (separate staging tiles -> the first
                # matmuls only wait for the first half)
                xv = xp[c]
                nc.vector.memset(xv[:, 0, :], 0.0)
                nc.vector.memset(xv[:, HP - 1, :], 0.0)
                nc.vector.memset(xv[:, 1 : HP - 1, 0], 0.0)
                nc.vector.memset(xv[:, 1 : HP - 1, WP - 1], 0.0)
                xs0 = stage.tile([128, 17, W], F32, name=f"xs{c}a", tag="xstage", bufs=4)
                nc.sync.dma_start(out=xs0, in_=x_d[c * 128 : (c + 1) * 128, 0:17, :])
                nc.vector.tensor_scalar_mul(xv[:, 1:18, 1 : W + 1], xs0, s1[c])
                xs1 = stage.tile([128, 15, W], F32, name=f"xs{c}b", tag="xstage", bufs=4)
                nc.sync.dma_start(out=xs1, in_=x_d[c * 128 : (c + 1) * 128, 17:32, :])
                nc.vector.tensor_scalar_mul(xv[:, 18:33, 1 : W + 1], xs1, s1[c])

                # w chunk: fp32 staging -> bf16 cast. Separate staging tiles
                # per DMA so each cast only depends on its own transfer;
                # chunk 0 (which gates the first matmul) loads per single pos.
                groups = (
                    [(p, p + 1) for p in range(1, KPOS)]
                    if c == 0
                    else [(3 * g, 3 * g + 3) for g in range(3)]
                )
                for lo, hi in groups:
                    wsg = stage.tile(
                        [128, hi - lo, COUT],
                        F32,
                        name=f"ws{c}_{lo}",
                        tag="wstage",
                        bufs=6,
                    )
                    nc.sync.dma_start(out=wsg, in_=w9_d[c, :, lo:hi, :])
                    for p in range(lo, hi):
                        nc.scalar.copy(w_sb[c][:, p, :], wsg[:, p - lo, :])


            # demod stats, emitted after all loads so their DVE ops never
            # get scheduled ahead of the x modulation: q = (1+s)^2 and
            # wsq[i,o] = sum_pos w^2 (square + contiguous add-tree).
            for c in range(NCH):
                nc.vector.tensor_mul(q[c], s1[c], s1[c])
                sq = sqpool.tile([128, KPOS, COUT], MMDT, name=f"sq{c}", tag="sq")
                nc.vector.tensor_mul(sq, w_sb[c], w_sb[c])
                for a, b in ((0, 1), (2, 3), (4, 5), (6, 7), (0, 2), (4, 6), (0, 4)):
                    nc.vector.tensor_add(sq[:, a, :], sq[:, a, :], sq[:, b, :])
                nc.vector.tensor_add(wsq[c], sq[:, 0, :], sq[:, 8, :])

            # --- conv: accumulate 36 matmuls into each of the 8 PSUM banks.
            acc = [
                [
                    psum.tile([128, 512], F32, name=f"acc{o}_{hh}", tag="acc")
                    for hh in range(2)
                ]
                for o in range(OCH)
            ]

            def conv_mm(c, o, pos):
                ky, kx = pos // 3, pos % 3
                lhsT = w_sb[c][:, pos, o * 128 : (o + 1) * 128]
                for hh in range(2):
                    rhs = xp[c][:, ky + hh * 16 : ky + hh * 16 + 16, kx : kx + 32]
                    nc.tensor.matmul(
                        acc[o][hh],
                        lhsT=lhsT,
                        rhs=rhs,
                        start=(c == 0 and pos == 0),
                        stop=(c == NCH - 1 and pos == KPOS - 1),
                    )

            # first chunks: pos-outer (matches w pos-group arrival order).
            # The very first position runs all hh=0 matmuls before hh=1 so
            # the PE starts as soon as the first x row-half is modulated.
            for hh in range(2):
                for o in range(OCH):
                    ky, kx = 0, 0
                    rhs = xp[0][:, hh * 16 : hh * 16 + 16, 0:32]
                    nc.tensor.matmul(
                        acc[o][hh],
                        lhsT=w_sb[0][:, 0, o * 128 : (o + 1) * 128],
                        rhs=rhs,
                        start=True,
                        stop=False,
                    )
            for c in range(NCH - 1):
                for pos in range(KPOS):
                    if c == 0 and pos == 0:
                        continue
                    for o in range(OCH):
                        conv_mm(c, o, pos)

            # Last chunk: cout-outer, so accumulators complete one cout chunk
            # at a time; drains / demod matvec / stores overlap the rest.
            for o in range(OCH):
                for pos in range(KPOS):
                    conv_mm(NCH - 1, o, pos)
                if o == 0:
                    # unscaled drain frees one PSUM bank for the demod matvec
                    nc.vector.tensor_copy(y_sb[0][:, 0:512], acc[0][0])
                    dsum = psum.tile([128, OCH], F32, name="dsum", tag="acc")
                    for oo in range(OCH):
                        for c in range(NCH):
                            nc.tensor.matmul(
                                dsum[:, oo : oo + 1],
                                lhsT=wsq[c][:, oo * 128 : (oo + 1) * 128],
                                rhs=q[c],
                                start=(c == 0),
                                stop=(c == NCH - 1),
                            )
                    nc.scalar.activation(
                        den_s, dsum, mybir.ActivationFunctionType.Sqrt, bias=eps_t
                    )
                    nc.vector.reciprocal(den, den_s)
                    # fix the unscaled half, scaled drain of the other half
                    nc.vector.tensor_scalar_mul(
                        y_sb[0][:, 0:512], y_sb[0][:, 0:512], den[:, 0:1]
                    )
                    nc.scalar.mul(y_sb[0][:, 512:1024], acc[0][1], den[:, 0:1])
                else:
                    # drains split across DVE and ACT so they run in parallel
                    nc.vector.tensor_scalar_mul(
                        y_sb[o][:, 0:512], acc[o][0], den[:, o : o + 1]
                    )
                    nc.scalar.mul(y_sb[o][:, 512:1024], acc[o][1], den[:, o : o + 1])
                # store per spatial half so each DMA starts as soon as its
                # half is drained; the halves go to different engines so the
                # two descriptor generations run in parallel
                nc.sync.dma_start(
                    out=y_d[o * 128 : (o + 1) * 128, 0:512], in_=y_sb[o][:, 0:512]
                )
                nc.scalar.dma_start(
                    out=y_d[o * 128 : (o + 1) * 128, 512:1024],
                    in_=y_sb[o][:, 512:1024],
                )

    nc.compile()
    return nc


def kernel(x, s, w):
    from concourse.bass_utils import run_bass_kernel_spmd

    global _compiled_nc
    if _compiled_nc is None:
        _compiled_nc = _build()
    nc = _compiled_nc

    x = np.asarray(x, dtype=np.float32)
    s = np.asarray(s, dtype=np.float32)
    w = np.asarray(w, dtype=np.float32)
    # w9[c, p, pos, o] = w[o, c*128+p, pos//3, pos%3]
    w9 = np.ascontiguousarray(np.transpose(w, (1, 2, 3, 0))).reshape(
        NCH, 128, KPOS, COUT
    )
    in_maps = [
        {
            "x": np.ascontiguousarray(x[i]),
            "s": np.ascontiguousarray(s[i].reshape(CIN, 1)),
            "w9": w9,
        }
        for i in range(B)
    ]
    res = run_bass_kernel_spmd(nc, in_maps, list(range(B))).results
    return np.stack([res[i]["y"].reshape(COUT, H, W) for i in range(B)], axis=0)
